# revision 8
# baseline (speedup 1.0000x reference)
"""Trainium2 Bass kernel for location-aware attention (ESPnet AttLoc).

Per batch row b:
    att_conv = conv1d(att_prev, conv_w)            # (C, T), 201-tap same-pad
    loc      = att_conv^T @ W_att                  # (T, D)
    enc_proj = X_b @ W_enc + b_enc                 # (T, D), X_b = enc_hs_pad[b]
    dec_proj = dec_z_b @ W_dec                     # (D,)
    e        = tanh(loc + enc_proj + dec_proj) @ gvec_w   # (T,)
    w        = softmax(2 * mask(e))                # (T,)
    c        = w @ X_b                             # (E,)

Sharding: data-parallel over batch across 8 cores (4 rows/core), weights
replicated. Per core, compute runs in transposed orientation (d on
partitions, t on free): PSUM accumulates W_enc^T X^T + W_att^T att_conv,
ACT fuses tanh with the per-partition bias (dec_proj + b_enc), the gvec
reduction is an M=1 matmul, softmax runs flat on partition 0, and the
weighted sum uses w transposed into PE columns against naturally-laid X.
X^T tiles are produced on-chip with PE transposes; the conv becomes a
matmul against a shifted-window matrix DMA'd from a zero-padded DRAM
staging of att_prev.

Matmul dtype: float32r runs the PE at 1 cycle/row vs 4 for float32. The
BIR verifier requires every producer feeding an fp32r matmul to emit
fp32r, so the whole matmul-operand chain (DRAM decls, DMA'd tiles, PE
transpose PSUM, DVE/ACT copies) is declared fp32r; fp32r bits are
identical to f32. memset can't emit fp32r, so zero/identity tiles are
built in f32 and DVE-copied over. ATT_MM_DT=f32 builds the exact-f32
variant (PE 4 cyc/row).
"""

import os
from contextlib import ExitStack

import numpy as np

import concourse.bass as bass
import concourse.tile as tile
from concourse import bacc, mybir, bass_utils
from concourse.masks import make_identity

# Problem shapes (hardcoded per contract)
B, T, E, D = 32, 2048, 1024, 1024
C = 10               # conv channels
J = 201              # conv taps
PAD = (J - 1) // 2   # 100
NCORES = 8
R = B // NCORES      # rows per core = 4
SCALING = 2.0

F32 = mybir.dt.float32
MM_DT = F32 if os.environ.get("ATT_MM_DT") == "f32" else mybir.dt.float32r

TC = T // 512        # 4 t-chunks of 512
EC = E // 128        # 8 e-chunks
DC = D // 128        # 8 d-chunks
PA = 2304            # padded att_prev length (max read: 128+127+2047=2302)


def _emit(ctx: ExitStack, tc: tile.TileContext, io: dict, mdt):
    nc = tc.nc
    x_d = io["x"]            # (R, T, E)      mdt
    len_d = io["lens"]       # (R,) int32
    decz_d = io["dec_z"]     # (R, D)         mdt
    attp_d = io["att_prev"]  # (R, T)         mdt
    wenc_d = io["W_enc"]     # (E, D)         mdt
    benc_d = io["b_enc"]     # (D,)           f32
    wdec_d = io["W_dec"]     # (D, D)         mdt
    watt_d = io["W_att"]     # (C, D)         mdt
    convw_d = io["conv_w"]   # (C, 1, J)      mdt
    gvec_d = io["gvec_w"]    # (D,)           mdt
    c_out = io["c_out"]      # (R, E)         f32
    w_out = io["w_out"]      # (R, T)         f32

    const = ctx.enter_context(tc.tile_pool(name="const", bufs=1))
    stage = ctx.enter_context(tc.tile_pool(name="stage", bufs=1))
    xpool = ctx.enter_context(tc.tile_pool(name="xp", bufs=2))
    xtpool = ctx.enter_context(tc.tile_pool(name="xtp", bufs=2))
    spool = ctx.enter_context(tc.tile_pool(name="sp", bufs=9))
    shpool = ctx.enter_context(tc.tile_pool(name="shp", bufs=1))
    rowp = ctx.enter_context(tc.tile_pool(name="rowp", bufs=1))
    dram = ctx.enter_context(tc.tile_pool(name="dram", bufs=1, space="DRAM"))

    ps_tp = ctx.enter_context(tc.tile_pool(name="ps_tp", bufs=2, space="PSUM"))
    ps_s = ctx.enter_context(tc.tile_pool(name="ps_s", bufs=2, space="PSUM"))
    ps_e = ctx.enter_context(tc.tile_pool(name="ps_e", bufs=2, space="PSUM"))
    ps_sm = ctx.enter_context(tc.tile_pool(name="ps_sm", bufs=2, space="PSUM"))

    # f32 identity (memset+affine_select are f32-only), copied to mdt
    identf = const.tile([128, 128], F32)
    make_identity(nc, identf)
    if mdt is F32:
        identm = identf
    else:
        identm = const.tile([128, 128], mdt)
        nc.vector.tensor_copy(out=identm, in_=identf)

    # ---- W_enc as lhsT chunks: we[p, ec, d] = W_enc[ec*128+p, d] ----
    we = const.tile([128, EC, D], mdt)
    nc.sync.dma_start(out=we, in_=wenc_d.rearrange("(ec p) d -> p ec d", p=128))

    # ---- W_att natural: wa[c, d] on partitions 0..9 ----
    wa = const.tile([16, D], mdt)
    nc.sync.dma_start(out=wa[0:C, :], in_=watt_d)

    # ---- gvec^T: gvecT[p, dc] = gvec[dc*128+p] ----
    gv_nat = stage.tile([1, D], F32, tag="stg")
    nc.sync.dma_start(out=gv_nat, in_=gvec_d.rearrange("(a d) -> a d", a=1))
    ps0 = ps_sm.tile([128, DC], F32, tag="sm")
    for mc in range(DC):
        nc.tensor.transpose(
            out=ps0[:, mc : mc + 1],
            in_=gv_nat[0:1, mc * 128 : (mc + 1) * 128],
            identity=identf[0:1, 0:1],
        )
    gvecT = const.tile([128, DC], mdt)
    nc.vector.tensor_copy(out=gvecT, in_=ps0)

    # ---- b_enc^T: bencT[p, dc] = b_enc[dc*128+p] (bias-only, f32) ----
    be_nat = stage.tile([1, D], F32, tag="stg")
    nc.sync.dma_start(out=be_nat, in_=benc_d.rearrange("(a d) -> a d", a=1))
    ps1 = ps_sm.tile([128, DC], F32, tag="sm")
    for mc in range(DC):
        nc.tensor.transpose(
            out=ps1[:, mc : mc + 1],
            in_=be_nat[0:1, mc * 128 : (mc + 1) * 128],
            identity=identf[0:1, 0:1],
        )
    bencT = const.tile([128, DC], F32)
    nc.vector.tensor_copy(out=bencT, in_=ps1)

    # ---- conv_w^T zero-padded to 256 taps: cwT[p, jc*C+c] = conv_w[c, jc*128+p] ----
    cv_nat = stage.tile([16, J], mdt, tag="stg")
    nc.sync.dma_start(out=cv_nat[0:C, :], in_=convw_d.rearrange("c one j -> (c one) j"))
    zf = const.tile([128, 2 * C], F32)
    nc.vector.memset(zf, 0.0)
    cwT = const.tile([128, 2 * C], mdt)
    nc.vector.tensor_copy(out=cwT, in_=zf)
    ps2 = ps_sm.tile([128, 2 * C], mdt, tag="sm")
    nc.tensor.transpose(
        out=ps2[:, 0:C], in_=cv_nat[0:C, 0:128], identity=identm[0:C, 0:C]
    )
    nc.tensor.transpose(
        out=ps2[0 : J - 128, C : 2 * C],
        in_=cv_nat[0:C, 128:J],
        identity=identm[0:C, 0:C],
    )
    nc.vector.tensor_copy(out=cwT[:, 0:C], in_=ps2[:, 0:C])
    nc.vector.tensor_copy(
        out=cwT[0 : J - 128, C : 2 * C], in_=ps2[0 : J - 128, C : 2 * C]
    )

    # ---- dec_z^T: dzT[p, k*R+r] = dec_z[r, k*128+p] ----
    dz_nat = stage.tile([R, D], mdt, tag="stg")
    nc.sync.dma_start(out=dz_nat, in_=decz_d)
    ps3 = ps_sm.tile([128, DC * R], mdt, tag="sm")
    for k in range(DC):
        nc.tensor.transpose(
            out=ps3[:, k * R : (k + 1) * R],
            in_=dz_nat[0:R, k * 128 : (k + 1) * 128],
            identity=identm[0:R, 0:R],
        )
    dzT = const.tile([128, DC * R], mdt)
    nc.vector.tensor_copy(out=dzT, in_=ps3)

    # ---- dec_proj = dec_z @ W_dec (natural [R, D]) ----
    dpa = ps_sm.tile([R, 512], F32, tag="sm")
    dpb = ps_sm.tile([R, 512], F32, tag="sm")
    dp_ps = [dpa, dpb]
    for k in range(DC):
        wd_k = xpool.tile([128, D], mdt, tag="xn")
        nc.sync.dma_start(out=wd_k, in_=wdec_d[k * 128 : (k + 1) * 128, :])
        for n in range(2):
            nc.tensor.matmul(
                dp_ps[n],
                lhsT=dzT[:, k * R : (k + 1) * R],
                rhs=wd_k[:, n * 512 : (n + 1) * 512],
                start=(k == 0),
                stop=(k == DC - 1),
            )
    dp_sb = const.tile([R, D], mdt)
    for n in range(2):
        nc.vector.tensor_copy(out=dp_sb[:, n * 512 : (n + 1) * 512], in_=dp_ps[n])

    # ---- bias_all[p, dc*R+r] = dec_proj[r, dc*128+p] + b_enc[dc*128+p] ----
    ps4 = ps_sm.tile([128, DC * R], mdt, tag="sm")
    for dc in range(DC):
        nc.tensor.transpose(
            out=ps4[:, dc * R : (dc + 1) * R],
            in_=dp_sb[0:R, dc * 128 : (dc + 1) * 128],
            identity=identm[0:R, 0:R],
        )
    bias_all = const.tile([128, DC * R], F32)
    for dc in range(DC):
        nc.vector.tensor_scalar_add(
            out=bias_all[:, dc * R : (dc + 1) * R],
            in0=ps4[:, dc * R : (dc + 1) * R],
            scalar1=bencT[:, dc : dc + 1],
        )

    # ---- zero-padded att_prev rows in DRAM ----
    pad_att = dram.tile([R, PA], mdt)
    zrow_f = const.tile([1, 256], F32)
    nc.vector.memset(zrow_f, 0.0)
    zrow = const.tile([1, 256], mdt)
    nc.vector.tensor_copy(out=zrow, in_=zrow_f)
    for r in range(R):
        nc.sync.dma_start(out=pad_att[r : r + 1, 0:PAD], in_=zrow[0:1, 0:PAD])
        nc.sync.dma_start(
            out=pad_att[r : r + 1, PAD + T : PA], in_=zrow[0:1, 0 : PA - PAD - T]
        )
        nc.sync.dma_start(out=pad_att[r : r + 1, PAD : PAD + T], in_=attp_d[r : r + 1, :])

    # ---- iota row (exact in f32) and per-row lengths as f32 ----
    iota_row = const.tile([1, T], F32)
    nc.gpsimd.iota(
        iota_row,
        pattern=[[1, T]],
        base=0,
        channel_multiplier=0,
        allow_small_or_imprecise_dtypes=True,
    )
    srow = const.tile([1, 16], F32)  # [len0..len3, rmax, nm2, sumexp, rinv]
    nc.gpsimd.dma_start(out=srow[0:1, 0:R], in_=len_d.rearrange("(a r) -> a r", a=1))

    # ================= per-row pipeline =================
    for r in range(R):
        # ---- shifted-window tiles of padded att_prev ----
        sh = []
        for jc in range(2):
            sh_t = shpool.tile([128, T], mdt, tag=f"sh{jc}")
            src = bass.AP(
                tensor=pad_att.tensor,
                offset=pad_att.offset + r * PA + jc * 128,
                ap=[[1, 128], [1, T]],
            )
            nc.sync.dma_start(out=sh_t, in_=src)
            sh.append(sh_t)
        # ---- att_conv[c, t] = sum_j conv_w[c, j] att_prev[t + j - PAD] ----
        ac_sb = shpool.tile([16, T], mdt, tag="ac")
        for t in range(TC):
            ac_ps = ps_sm.tile([16, 512], F32, tag="sm")
            for jc in range(2):
                nc.tensor.matmul(
                    ac_ps[0:C, :],
                    lhsT=cwT[:, jc * C : (jc + 1) * C],
                    rhs=sh[jc][:, t * 512 : (t + 1) * 512],
                    start=(jc == 0),
                    stop=(jc == 1),
                )
            nc.scalar.copy(out=ac_sb[0:C, t * 512 : (t + 1) * 512], in_=ac_ps[0:C, :])

        e_row = rowp.tile([1, T], F32, tag="e_row")

        for t in range(TC):
            # ---- X natural: xn[p, a, e] = X[t*512 + a*128 + p, e] ----
            xn = xpool.tile([128, 4, E], mdt, tag="xn")
            nc.sync.dma_start(
                out=xn,
                in_=x_d[r, t * 512 : (t + 1) * 512, :].rearrange(
                    "(a p) e -> p a e", p=128
                ),
            )
            # ---- PE transpose: xt[p, ec, tl] = X[t*512 + tl, ec*128 + p] ----
            xt = xtpool.tile([128, EC, 512], mdt, tag="xt")
            for ec in range(EC):
                tp = ps_tp.tile([128, 512], mdt, tag="tp")
                for a in range(4):
                    nc.tensor.transpose(
                        out=tp[:, a * 128 : (a + 1) * 128],
                        in_=xn[:, a, ec * 128 : (ec + 1) * 128],
                        identity=identm,
                    )
                nc.vector.tensor_copy(out=xt[:, ec, :], in_=tp)
            # ---- (enc_proj + loc)^T accumulation + fused tanh ----
            s_tiles = []
            for dc in range(DC):
                sp = ps_s.tile([128, 512], F32, tag="ps_s")
                for ec in range(EC):
                    nc.tensor.matmul(
                        sp,
                        lhsT=we[:, ec, dc * 128 : (dc + 1) * 128],
                        rhs=xt[:, ec, :],
                        start=(ec == 0),
                        stop=False,
                    )
                nc.tensor.matmul(
                    sp,
                    lhsT=wa[0:C, dc * 128 : (dc + 1) * 128],
                    rhs=ac_sb[0:C, t * 512 : (t + 1) * 512],
                    start=False,
                    stop=True,
                )
                s_sb = spool.tile([128, 512], mdt, tag="s")
                nc.scalar.activation(
                    out=s_sb,
                    in_=sp,
                    func=mybir.ActivationFunctionType.Tanh,
                    bias=bias_all[:, dc * R + r : dc * R + r + 1],
                    scale=1.0,
                )
                s_tiles.append(s_sb)
            # ---- e chunk: gvec reduction (deferred so PE stays dense) ----
            eps_t = ps_e.tile([1, 512], F32, tag="eps")
            for dc in range(DC):
                nc.tensor.matmul(
                    eps_t,
                    lhsT=gvecT[:, dc : dc + 1],
                    rhs=s_tiles[dc],
                    start=(dc == 0),
                    stop=(dc == DC - 1),
                )
            nc.vector.tensor_copy(
                out=e_row[0:1, t * 512 : (t + 1) * 512], in_=eps_t
            )

        # ---- masked softmax over t (flat on partition 0) ----
        amask = rowp.tile([1, T], F32, tag="amask")
        nc.vector.tensor_scalar(
            out=amask,
            in0=iota_row,
            scalar1=srow[0:1, r : r + 1],
            scalar2=-1.0e30,
            op0=mybir.AluOpType.is_ge,
            op1=mybir.AluOpType.mult,
        )
        nc.vector.tensor_add(out=e_row, in0=e_row, in1=amask)
        nc.vector.tensor_reduce(
            out=srow[0:1, 4:5],
            in_=e_row,
            axis=mybir.AxisListType.X,
            op=mybir.AluOpType.max,
        )
        nc.vector.tensor_scalar_mul(
            out=srow[0:1, 5:6], in0=srow[0:1, 4:5], scalar1=-SCALING
        )
        nc.scalar.activation(
            out=e_row,
            in_=e_row,
            func=mybir.ActivationFunctionType.Exp,
            bias=srow[0:1, 5:6],
            scale=SCALING,
            accum_out=srow[0:1, 6:7],
        )
        nc.vector.reciprocal(out=srow[0:1, 7:8], in_=srow[0:1, 6:7])
        nc.vector.tensor_scalar_mul(out=e_row, in0=e_row, scalar1=srow[0:1, 7:8])
        nc.sync.dma_start(out=w_out[r : r + 1, :], in_=e_row)

        # ---- w into PE columns: w16[p, i] = w[i*128 + p] ----
        wps = ps_sm.tile([128, 16], F32, tag="sm")
        for i in range(16):
            nc.tensor.transpose(
                out=wps[:, i : i + 1],
                in_=e_row[0:1, i * 128 : (i + 1) * 128],
                identity=identf[0:1, 0:1],
            )
        w16 = rowp.tile([128, 16], mdt, tag="w16")
        nc.vector.tensor_copy(out=w16, in_=wps)

        # ---- c = w @ X against re-streamed natural X ----
        cpa = ps_sm.tile([1, 512], F32, tag="sm")
        cpb = ps_sm.tile([1, 512], F32, tag="sm")
        c_ps = [cpa, cpb]
        c_sb = rowp.tile([1, E], F32, tag="c_sb")
        for t in range(TC):
            xc = xpool.tile([128, 4, E], mdt, tag="xn")
            nc.sync.dma_start(
                out=xc,
                in_=x_d[r, t * 512 : (t + 1) * 512, :].rearrange(
                    "(a p) e -> p a e", p=128
                ),
            )
            for a in range(4):
                i = t * 4 + a
                for en in range(2):
                    nc.tensor.matmul(
                        c_ps[en],
                        lhsT=w16[:, i : i + 1],
                        rhs=xc[:, a, en * 512 : (en + 1) * 512],
                        start=(i == 0),
                        stop=(i == 15),
                    )
        for en in range(2):
            nc.vector.tensor_copy(
                out=c_sb[0:1, en * 512 : (en + 1) * 512], in_=c_ps[en]
            )
        nc.sync.dma_start(out=c_out[r : r + 1, :], in_=c_sb)


def build(mdt=None):
    if mdt is None:
        mdt = MM_DT
    nc = bacc.Bacc("TRN2", target_bir_lowering=False, debug=False)
    io = {
        "x": nc.dram_tensor("x", [R, T, E], mdt, kind="ExternalInput").ap(),
        "lens": nc.dram_tensor("lens", [R], mybir.dt.int32, kind="ExternalInput").ap(),
        "dec_z": nc.dram_tensor("dec_z", [R, D], mdt, kind="ExternalInput").ap(),
        "att_prev": nc.dram_tensor("att_prev", [R, T], mdt, kind="ExternalInput").ap(),
        "W_enc": nc.dram_tensor("W_enc", [E, D], mdt, kind="ExternalInput").ap(),
        "b_enc": nc.dram_tensor("b_enc", [D], F32, kind="ExternalInput").ap(),
        "W_dec": nc.dram_tensor("W_dec", [D, D], mdt, kind="ExternalInput").ap(),
        "W_att": nc.dram_tensor("W_att", [C, D], mdt, kind="ExternalInput").ap(),
        "conv_w": nc.dram_tensor("conv_w", [C, 1, J], mdt, kind="ExternalInput").ap(),
        "gvec_w": nc.dram_tensor("gvec_w", [D], F32, kind="ExternalInput").ap(),
        "c_out": nc.dram_tensor("c_out", [R, E], F32, kind="ExternalOutput").ap(),
        "w_out": nc.dram_tensor("w_out", [R, T], F32, kind="ExternalOutput").ap(),
    }
    with tile.TileContext(nc) as tcx:
        with ExitStack() as ctx:
            _emit(ctx, tcx, io, mdt)
    nc.compile()
    return nc


_built = None


def _get_nc():
    global _built
    if _built is None:
        _built = build()
    return _built


def make_in_maps(inputs):
    f = lambda a: np.ascontiguousarray(np.asarray(a), dtype=np.float32)
    x = f(inputs["enc_hs_pad"])
    lens = np.ascontiguousarray(np.asarray(inputs["enc_hs_len"]), dtype=np.int32)
    dec_z = f(inputs["dec_z"])
    att_prev = f(inputs["att_prev"])
    shared = {
        "W_enc": f(inputs["W_enc"]),
        "b_enc": f(inputs["b_enc"]),
        "W_dec": f(inputs["W_dec"]),
        "W_att": f(inputs["W_att"]),
        "conv_w": f(inputs["conv_w"]),
        "gvec_w": f(inputs["gvec_w"]),
    }
    in_maps = []
    for core in range(NCORES):
        sl = slice(core * R, (core + 1) * R)
        m = dict(shared)
        m["x"] = np.ascontiguousarray(x[sl])
        m["lens"] = np.ascontiguousarray(lens[sl])
        m["dec_z"] = np.ascontiguousarray(dec_z[sl])
        m["att_prev"] = np.ascontiguousarray(att_prev[sl])
        in_maps.append(m)
    return in_maps


def kernel(**inputs):
    nc = _get_nc()
    in_maps = make_in_maps(inputs)
    res = bass_utils.run_bass_kernel_spmd(nc, in_maps, core_ids=list(range(NCORES)))
    c = np.concatenate([res.results[i]["c_out"] for i in range(NCORES)], axis=0)
    w = np.concatenate([res.results[i]["w_out"] for i in range(NCORES)], axis=0)
    return c, w
